# revision 35
# baseline (speedup 1.0000x reference)
"""Chamfer distance loss on 8 Trainium2 NeuronCores.

Full inputs: points1 [16, 4096, 3], points2 [16, 4096, 3] (fp32).
Output: scalar fp32 loss = (sum(min_m dist) + sum(min_n dist)) / B.

Sharding: data-parallel over batch B=16 -> 2 batches per core on 8 cores.
Each core computes a partial scalar (sum of row-mins + col-mins for its
batches); host sums the 8 partials and divides by B.

Per-batch device algorithm (per core), v3:
  dist[n, m] = |a_n|^2 + |b_m|^2 - 2 a.b  computed as one K=7 matmul:
    psum = matmul(lhsT=[ax,ay,az,-.5,-.5,-.5,-|a|^2/2],
                  rhs =[bx,by,bz,bx^2,by^2,bz^2,1])
         = a.b - |b|^2/2 - |a|^2/2 = -dist/2   (fp32r, 512 cols per bank)
    dist16 = ScalarE Identity(psum * -2)       (bias-free evacuation)
  All matmul operands (coords, consts, squares, |a|^2 row) are PRECOMPUTED
  ON HOST, DMA'd in as the 7 unique rows, then replicated on-device to the
  4 PE row-group offsets with SBUF-to-SBUF DMAs.
  Stripes (128 rows of n) are processed in QUADS of 4; the bf16 dist tiles
  of a quad live in one ring tile [128, 4, 4096] so the row-min fold tree
  runs as ONE DVE op per level over all 4 stripes ([128, 4, w] 3D APs) -
  DVE per-op overhead dominated v1. DVE ops stay <= 2048 elems/partition
  wide (wider flat ops hit a slow path).
  col-min: DVE tensor_tensor min into acc per stripe; final col-min across
  partitions via PE transpose + strided reduce-min; row+col sums via one
  merged reduce-add + one matmul with ones.

v4/v5 (this file): input loads for both batches prefetched up front;
  batch tails (transpose phase) deferred after all main loops so they
  overlap the next batch's stream; fold tree extended to w=32; DVE op
  count minimized (HW shows ~200-400ns/op hidden overhead vs the
  timeline sim): fold tree batched over 8 stripes ([128,8,w] t01), col
  accumulate as one [128,2,2048] 3D op per stripe, fold1 as one
  [128,2,2048] op per stripe PAIR, both batches' finals merged into a
  single reduce+matmul+copy, out-DMA on the Activation HWDGE queue so
  the SP queue prefetches the next iteration's loads early. Keep inner
  AP dims <= 2048 (2x_1P fast path); >2048 total width in 3D is fine.
  Offload experiments that are DEAD ENDS on this stack (kept as inert
  flags): SDMA CCE accum-min (HWDGE silently ignores cce_op - measured
  dst==src; SWDGE whitelist is add-only) and GpSimd tensor_tensor
  (NCC_IXCG966: TT opcode invalid on Pool engine for TRN2 CoreV3).
  Engine budget per batch (timeline sim): DVE ~150us (col TTs 72 +
  fold1 36 + tree 31 + reduces 10) is the bottleneck at ~94% busy;
  ACT evacuation ~121us; PE matmuls ~58us. DVE work is at the
  structural floor for this dataflow (every dist element must pass
  through exactly one bf16 TT for the col path and one for the row
  fold; TT caps at 2x_1P on cayman).
"""

import time

import numpy as np

import concourse.bacc as bacc
import concourse.mybir as mybir
import concourse.tile as tile
from concourse import bass_utils
from concourse.masks import make_identity

N_CORES = 8

f32 = mybir.dt.float32
f32r = mybir.dt.float32r
f16 = mybir.dt.bfloat16
AF = mybir.ActivationFunctionType
ALU = mybir.AluOpType
AX = mybir.AxisListType

_CACHE = {}
last_exec_seconds = None  # wall time of the device dispatch (set per call)

QUAD = 4         # stripes per quad (ring depth)
K7 = True        # fold |a|^2 into the matmul (K=7) -> bias-free activations
PSW = 2048       # psum group width (2048 | 4096)
PS_BUFS = 2      # psum pool bufs (PSW//512 banks each; total <= 8 banks)
RG = 4           # PE row-groups for concurrent matmuls (1 | 2 | 4)
EVAC_ON = True   # timing attribution: ScalarE evacuation
ROWMIN_ON = True  # timing attribution: t01 + quad fold tree
COLMIN_ON = True  # timing attribution: colacc TTs
M_DMA = 0        # col-min cols offloaded to SDMA-CCE accum-min chains (0=off)
                 # NOTE: dead end on this stack — HWDGE silently ignores
                 # cce_op (measured: dst==src copy), SWDGE only allows add.
DMA_CHAINS = 2   # independent accD RMW chains (parallel sub-chunks of M_DMA)
GP_COLS = 0    # col-min cols offloaded to GpSimd TT-min chains (0=off);
                 # measures whether the shared POOL SBUF port makes this
                 # net-negative vs DVE doing everything.

KDIM = 7 if K7 else 6
NROWS = 32 * (RG - 1) + KDIM


def _build(bl: int, n: int, m: int, repeat: int = 1):
    """Build the SPMD module for bl batches of [n x 3] vs [m x 3] points.

    repeat > 1 wraps the whole computation in a hardware For_i loop that
    recomputes the same result `repeat` times — used only for timing.
    """
    assert n % (128 * QUAD) == 0 and m % PSW == 0
    n_stripes = n // 128
    n_quads = n_stripes // QUAD
    n_groups = m // PSW

    nc = bacc.Bacc("TRN2", target_bir_lowering=False, debug=False)
    a6d = nc.dram_tensor("a6d", [bl, KDIM, n], f32r, kind="ExternalInput")
    b6d = nc.dram_tensor("b6d", [bl, KDIM, m], f32r, kind="ExternalInput")
    if not K7:
        a2d = nc.dram_tensor("a2d", [bl, 128, n // 128], f32, kind="ExternalInput")
    out = nc.dram_tensor("out", [1, 1], f32, kind="ExternalOutput")

    with tile.TileContext(nc) as tc:
        with (
            tc.tile_pool(name="const", bufs=1) as constp,
            tc.tile_pool(name="pts", bufs=2) as ptsp,
            tc.tile_pool(name="acc", bufs=2) as accp,
            tc.tile_pool(name="ring", bufs=2) as ringp,
            tc.tile_pool(name="t01", bufs=1) as t01p,
            tc.tile_pool(name="small", bufs=4) as smallp,
            tc.tile_pool(name="psum", bufs=PS_BUFS, space="PSUM") as psump,
        ):
            ident = constp.tile([128, 128], f16)
            make_identity(nc, ident[:])
            ones128 = constp.tile([128, 1], f32)
            nc.gpsimd.memset(ones128[:], 1.0)
            out_sb = constp.tile([1, 1], f32)

            import contextlib
            import os
            unroll = int(os.environ.get("BASS_SIM_UNROLL", "0"))
            loop_ctx = (
                tc.For_i(0, repeat, 1)
                if repeat > 1 and not unroll
                else contextlib.nullcontext()
            )
            outer_reps = repeat if (repeat > 1 and unroll) else 1
            m_dve = m - M_DMA - GP_COLS
            msz = n_stripes + m // 128
            # persistent across For_i iterations (needed by tail rotation)
            minsall = constp.tile([128, bl * msz], f16, name="minsall")
            rot = False  # rotation measured worse: psum 2-slot pool re-couples
            if rot:
                assert M_DMA == 0 and GP_COLS == 0

            def emit_loads():
                ab_tiles = []
                for b in range(bl):
                    a6 = ptsp.tile([NROWS, n], f32r, tag="a6", name="a6")
                    b6 = ptsp.tile([NROWS, m], f32r, tag="b6", name="b6")
                    nc.sync.dma_start(a6[0:KDIM, :], a6d.ap()[b])
                    nc.sync.dma_start(b6[0:KDIM, :], b6d.ap()[b])
                    for rg in range(1, RG):
                        nc.sync.dma_start(
                            a6[32 * rg : 32 * rg + KDIM, :], a6[0:KDIM, :]
                        )
                        nc.sync.dma_start(
                            b6[32 * rg : 32 * rg + KDIM, :], b6[0:KDIM, :]
                        )
                    ab_tiles.append((a6, b6))
                return ab_tiles

            def emit_main(b, a6, b6, acc):
                mins = minsall[:, b * msz : (b + 1) * msz]
                t01 = None
                for q in range(n_quads):
                    ring = ringp.tile([128, QUAD, m], f16, tag="ring", name="ring")
                    if q % 2 == 0:
                        # t01 spans TWO quads: fold tree once per oct with
                        # [128, 8, w] APs (fewest DVE ops)
                        t01 = t01p.tile(
                            [128, 2 * QUAD, m // 2], f16, tag="t01", name="t01"
                        )
                    for si in range(QUAD):
                        s = q * QUAD + si
                        ssl = slice(128 * s, 128 * (s + 1))
                        for g in range(n_groups):
                            ps = psump.tile([128, PSW], f32, tag="mm", name="ps")
                            for j in range(PSW // 512):
                                mo = PSW * g + 512 * j
                                ro = 32 * ((g * (PSW // 512) + j) % RG)
                                nc.tensor.matmul(
                                    ps[:, 512 * j : 512 * (j + 1)],
                                    a6[ro : ro + KDIM, ssl],
                                    b6[ro : ro + KDIM, mo : mo + 512],
                                    start=True,
                                    stop=True,
                                    tile_position=(ro, 0),
                                )
                            gsl = slice(PSW * g, PSW * (g + 1))
                            nc.scalar.activation(
                                ring[:, si, gsl], ps[:], AF.Identity,
                                bias=0.0, scale=-2.0,
                            )
                        # col-min: one 3D op [128, c, 2048] per stripe
                        # (inner AP dim <= 2048 stays on the 2x_1P path)
                        nch = m_dve // 2048
                        a3 = acc[:, 0 : nch * 2048].rearrange(
                            "p (c x) -> p c x", x=2048
                        )
                        r3 = ring[:, si, 0 : nch * 2048].rearrange(
                            "p (c x) -> p c x", x=2048
                        )
                        if s == 0:
                            nc.vector.tensor_copy(a3, r3)
                        else:
                            nc.vector.tensor_tensor(a3, a3, r3, ALU.min)
                        # first fold m -> m/2: one [128, 2, 2048] 3D op per
                        # PAIR of stripes
                        if si % 2 == 1:
                            oi = (q % 2) * QUAD + si
                            nc.vector.tensor_tensor(
                                t01[:, oi - 1 : oi + 1, :],
                                ring[:, si - 1 : si + 1, 0 : m // 2],
                                ring[:, si - 1 : si + 1, m // 2 : m],
                                ALU.min,
                            )
                    # oct-batched fold tree: one op per level, 8 stripes
                    if q % 2 == 1:
                        w = m // 4
                        while w >= 32:
                            nc.vector.tensor_tensor(
                                t01[:, :, 0:w], t01[:, :, 0:w],
                                t01[:, :, w : 2 * w], ALU.min,
                            )
                            w //= 2
                        nc.vector.tensor_reduce(
                            mins[:, (q - 1) * QUAD : (q + 1) * QUAD],
                            t01[:, :, 0:32],
                            axis=AX.X,
                            op=ALU.min,
                        )

            def emit_tail(b, acc):
                # col-min across partitions: 32 PE transposes into ONE f16
                # psum tile (8KB - same mm slot size), then a single strided
                # reduce-min. One psum alloc per tail keeps the matmul psum
                # rotation decoupled from tail reduces.
                mins = minsall[:, b * msz : (b + 1) * msz]
                n_blocks = m // 128
                pst = psump.tile([128, 2 * PSW], f16, tag="mm", name="pst")
                for k in range(n_blocks):
                    nc.tensor.transpose(
                        pst[:, 128 * k : 128 * (k + 1)],
                        acc[:, 128 * k : 128 * (k + 1)],
                        ident[:],
                    )
                nc.vector.tensor_reduce(
                    mins[:, n_stripes : n_stripes + n_blocks],
                    pst[:].rearrange("p (k x) -> p k x", x=128),
                    axis=AX.X,
                    op=ALU.min,
                )
                return pst

            def emit_final(pst):
                # merged sum over BOTH batches' mins -> scalar -> out.
                # The ones-matmul reuses the tail's pst psum tile (bitcast
                # f32 corner) - no extra end-of-iteration psum allocation to
                # WAR-block the next iteration's first matmuls.
                tsum = smallp.tile([128, 1], f32, tag="tsum", name="tsum")
                nc.vector.tensor_reduce(
                    tsum[:], minsall[:], axis=AX.X, op=ALU.add
                )
                sc = pst[0:1, 0:2].bitcast(f32)
                nc.tensor.matmul(
                    sc[0:1, 0:1], tsum[:], ones128[:], start=True, stop=True
                )
                nc.vector.tensor_copy(out_sb[0:1, 0:1], sc[0:1, 0:1])
                # out DMA on the ACT HWDGE queue: keeps SP free for prefetch
                nc.scalar.dma_start(out.ap(), out_sb[:])

            acc_b1 = None
            with loop_ctx:
                for _rep in range(outer_reps):
                    if rot:
                        # rotated body: emit prev iteration's b1-tail + final
                        # FIRST - their DVE work fills the iteration-start
                        # bubble while the new loads/matmuls/evacs ramp up.
                        # acc_b1 is allocated up top so the pool hands the
                        # same buffer every iteration (2 allocs/body, bufs=2);
                        # iteration 0 computes garbage, overwritten later,
                        # and the post-loop epilogue emits the true final.
                        acc_b1 = accp.tile(
                            [128, m_dve], f16, tag="acc", name="acc_b1"
                        )
                        pst = emit_tail(1, acc_b1)
                        emit_final(pst)
                        ab = emit_loads()
                        acc_b0 = accp.tile(
                            [128, m_dve], f16, tag="acc", name="acc_b0"
                        )
                        emit_main(0, ab[0][0], ab[0][1], acc_b0)
                        emit_tail(0, acc_b0)
                        emit_main(1, ab[1][0], ab[1][1], acc_b1)
                    else:
                        ab = emit_loads()
                        pst = None
                        for b in range(bl):
                            a = accp.tile(
                                [128, m_dve], f16, tag="acc", name="acc"
                            )
                            emit_main(b, ab[b][0], ab[b][1], a)
                            # tail immediately after each main: b0's pst is
                            # then read mid-iteration, so the next
                            # iteration's first matmul psum slot (same
                            # rotation parity) is free early.
                            pst = emit_tail(b, a)
                        emit_final(pst)
            if rot:
                pst = emit_tail(1, acc_b1)
                emit_final(pst)

    nc.finalize()
    return nc


def _prep(points, bl):
    """Host-side: [B, N, 3] fp32 -> per-core lhsT/rhs arrays + |a|^2 bias.

    Returns (x6 [B, NROWS, N], x2c [B, 128, N//128]) where x6 rows
    32*rg + (0..5) = [x, y, z, -0.5, -0.5, -0.5] replicated for each PE
    row-group, and rhs rows 3..5 hold the squared coords instead of -0.5
    (the b-side). The caller picks which rows matter.
    """
    B, N, _ = points.shape
    xT = points.transpose(0, 2, 1)  # [B, 3, N]
    x6 = np.zeros((B, KDIM, N), dtype=np.float32)
    sq = xT * xT
    x2 = sq.sum(axis=1)  # [B, N]
    x6[:, 0:3] = xT
    # (replication to NROWS happens in _in_maps after rows 3.. are filled)
    x2c = np.ascontiguousarray(
        x2.reshape(B, N // 128, 128).transpose(0, 2, 1)
    )  # [B, 128, N//128], x2c[b, p, s] = |x_{128 s + p}|^2
    return x6, sq, x2, x2c


def _in_maps(points1, points2):
    points1 = np.ascontiguousarray(np.asarray(points1), dtype=np.float32)
    points2 = np.ascontiguousarray(np.asarray(points2), dtype=np.float32)
    btot = points1.shape[0]
    bl = btot // N_CORES
    a6, _, a2, a2c = _prep(points1, bl)
    b6, bsq, _, _ = _prep(points2, bl)
    # a-side rows 3:5 = -0.5 consts; b-side rows 3:5 = squared coords
    a6[:, 3:6] = -0.5
    b6[:, 3:6] = bsq
    if K7:
        # psum = a.b - |b|^2/2 - |a|^2/2 = -dist/2; act scale=-2, no bias
        a6[:, 6] = -0.5 * a2
        b6[:, 6] = 1.0
    maps = [
        {
            "a6d": a6[c * bl : (c + 1) * bl],
            "b6d": b6[c * bl : (c + 1) * bl],
        }
        for c in range(N_CORES)
    ]
    if not K7:
        for c in range(N_CORES):
            maps[c]["a2d"] = a2c[c * bl : (c + 1) * bl]
    return maps


def kernel(points1, points2):
    global last_exec_seconds
    points1 = np.ascontiguousarray(np.asarray(points1), dtype=np.float32)
    points2 = np.ascontiguousarray(np.asarray(points2), dtype=np.float32)
    btot, n, _ = points1.shape
    m = points2.shape[1]
    bl = btot // N_CORES

    key = (bl, n, m)
    if _CACHE.get("key") != key:
        _CACHE["nc"] = _build(bl, n, m)
        _CACHE["key"] = key
    nc = _CACHE["nc"]

    in_maps = _in_maps(points1, points2)
    t0 = time.time()
    res = bass_utils.run_bass_kernel_spmd(
        nc, in_maps, core_ids=list(range(N_CORES))
    )
    last_exec_seconds = time.time() - t0

    total = np.float64(0.0)
    for r in res.results:
        total += r["out"].astype(np.float64).sum()
    return np.float32(total / btot)

